# revision 4
# baseline (speedup 1.0000x reference)
"""Trainium2 Bass kernel for nn_BinaryDecorator: y = (sign(x) @ sign(W).T + b) * mean(|x|).

Full shapes: x [16384, 4096] f32, W [4096, 4096] f32, b [4096] f32 -> y [16384, 4096] f32.

Strategy (data-parallel over tokens, 8 cores):
  - host: shard x by tokens (2048 rows/core); replicate W (pre-transposed to WT=[K,N],
    a pure layout choice for the replicated weight) and b.
  - device, per core:
      phase 1: stream x f32, ACT Sign -> bf16, DVE abs-reduce partial sum(|x|),
               PE-transpose 128x128 sign tiles, evict to resident fp8 sxT [ki, kb, t].
      mean:    cross-partition reduce (gpsimd), AllReduce scalar across 8 cores,
               scale 1/N -> per-partition mean; meanb = b * mean (b broadcast to 128 parts).
      phase 2: stream WT f32 per 1024-col super, ACT Sign -> fp8 swT [ki, kb, n];
               fp8 DoubleRow matmuls accumulate over K in PSUM;
               evict with one fused DVE op: y = psum * mean + meanb.
  - host: concatenate the 8 token shards.

sign() values are exactly representable in fp8e4m3 and the K<=4096 integer sums are
exact in fp32 PSUM, so the matmul is bit-exact; only the mean reduction rounds.
"""

import numpy as np

P = 128
MM_N = 512  # psum free width per matmul


def _build_nc(T, K, N, n_cores, n_tot):
    """Build + compile the per-core Bass program. T,K,N are per-core shard dims;
    n_tot is the global element count of x (for the mean)."""
    import concourse.bacc as bacc
    import concourse.bass_isa as bass_isa
    import concourse.mybir as mybir
    import concourse.tile as tile
    from concourse.masks import make_identity

    f32 = mybir.dt.float32
    bf16 = mybir.dt.bfloat16
    fp8 = mybir.dt.float8e4

    assert T % P == 0 and K % (2 * P) == 0 and N % MM_N == 0
    TB = T // P  # token blocks
    KB = K // P  # contraction blocks
    KC = KB // 2  # DoubleRow k-pair count
    NW = min(1024, N)  # n columns per super-tile
    NS = N // NW  # super-tiles
    NQ = NW // MM_N  # psum chunks per super
    XW = min(2048, K)  # x columns per load chunk
    NH = K // XW  # x chunks per token block

    nc = bacc.Bacc(
        "TRN2",
        target_bir_lowering=False,
        debug=False,
        enable_asserts=False,
        num_devices=n_cores,
    )
    x = nc.dram_tensor("x", [T, K], f32, kind="ExternalInput").ap()
    wt = nc.dram_tensor("WT", [K, N], f32, kind="ExternalInput").ap()
    bvec = nc.dram_tensor("b", [1, N], f32, kind="ExternalInput").ap()
    y = nc.dram_tensor("y", [T, N], f32, kind="ExternalOutput").ap()

    with tile.TileContext(nc) as tc:
        with (
            tc.tile_pool(name="const", bufs=1) as constp,
            tc.tile_pool(name="xin", bufs=2) as xinp,
            tc.tile_pool(name="sxb", bufs=2) as sxbp,
            tc.tile_pool(name="wf", bufs=2) as wfp,
            tc.tile_pool(name="sxT", bufs=1) as sxTp,
            tc.tile_pool(name="swT", bufs=2) as swTp,
            tc.tile_pool(name="yout", bufs=2) as youtp,
            tc.tile_pool(name="pst", bufs=4, space="PSUM") as pstp,
            tc.tile_pool(name="psmm", bufs=2, space="PSUM") as psmmp,
            tc.tile_pool(name="dram", bufs=1, space="DRAM") as dramp,
        ):
            ident = constp.tile([P, P], bf16)
            make_identity(nc, ident)

            accs = constp.tile([P, TB * NH], f32)
            sxT = sxTp.tile([P, KB, T], fp8)

            # ---- phase 1: binarize + transpose x, accumulate sum(|x|) ----
            for tb in range(TB):
                for h in range(NH):
                    xin = xinp.tile([P, XW], f32)
                    nc.sync.dma_start(
                        xin, x[tb * P : (tb + 1) * P, h * XW : (h + 1) * XW]
                    )
                    sxb = sxbp.tile([P, XW], bf16)
                    nc.scalar.sign(sxb, xin)
                    nc.vector.tensor_reduce(
                        out=accs[:, tb * NH + h : tb * NH + h + 1],
                        in_=xin,
                        op=mybir.AluOpType.add,
                        axis=mybir.AxisListType.X,
                        apply_absolute_value=True,
                    )
                    for j in range(XW // P):
                        kb = h * (XW // P) + j
                        pst = pstp.tile([P, P], bf16)
                        nc.tensor.transpose(pst, sxb[:, j * P : (j + 1) * P], ident)
                        nc.vector.tensor_copy(
                            out=sxT[:, kb, tb * P : (tb + 1) * P], in_=pst
                        )

            # ---- mean(|x|) across all cores ----
            acc1 = constp.tile([P, 1], f32)
            nc.vector.tensor_reduce(
                out=acc1, in_=accs, op=mybir.AluOpType.add, axis=mybir.AxisListType.X
            )
            allred = constp.tile([P, 1], f32)
            nc.gpsimd.partition_all_reduce(
                allred, acc1, channels=P, reduce_op=bass_isa.ReduceOp.add
            )
            stg = constp.tile([1, 16], f32)
            nc.vector.memset(stg, 0.0)
            nc.vector.tensor_copy(out=stg[0:1, 0:1], in_=allred[0:1, :])
            cc_in = dramp.tile([1, 16], f32)
            cc_out = dramp.tile([1, 16], f32)
            nc.sync.dma_start(cc_in[:], stg[:])
            nc.gpsimd.collective_compute(
                "AllReduce",
                mybir.AluOpType.add,
                replica_groups=[list(range(n_cores))],
                ins=[cc_in.opt()],
                outs=[cc_out.opt()],
            )
            tot = constp.tile([1, 1], f32)
            nc.sync.dma_start(tot[:], cc_out[0:1, 0:1])
            mean_sc = constp.tile([P, 1], f32)
            nc.gpsimd.partition_broadcast(mean_sc, tot[:], channels=P)
            nc.vector.tensor_scalar(
                out=mean_sc,
                in0=mean_sc,
                scalar1=float(1.0 / n_tot),
                scalar2=None,
                op0=mybir.AluOpType.mult,
            )

            # ---- meanb[p, n] = b[n] * mean (b broadcast across partitions) ----
            b_sb = constp.tile([1, N], f32)
            nc.sync.dma_start(b_sb[:], bvec[0:1, :])
            meanb = constp.tile([P, N], f32)
            nc.gpsimd.partition_broadcast(meanb, b_sb[:], channels=P)
            nc.vector.tensor_scalar(
                out=meanb,
                in0=meanb,
                scalar1=mean_sc[:, 0:1],
                scalar2=None,
                op0=mybir.AluOpType.mult,
            )

            # ---- phase 2: stream W, fp8 DoubleRow matmul, fused eviction ----
            for s in range(NS):
                swT = swTp.tile([P, KB, NW], fp8)
                for kb in range(KB):
                    wf = wfp.tile([P, NW], f32)
                    nc.sync.dma_start(
                        wf, wt[kb * P : (kb + 1) * P, s * NW : (s + 1) * NW]
                    )
                    nc.scalar.sign(swT[:, kb, :], wf)
                for tb in range(TB):
                    pss = [
                        psmmp.tile([P, MM_N], f32, name=f"ps{q}", tag=f"psmm{q}")
                        for q in range(NQ)
                    ]
                    for kc in range(KC):
                        lhs = sxT[:, 2 * kc : 2 * kc + 2, tb * P : (tb + 1) * P]
                        for q in range(NQ):
                            nc.tensor.matmul(
                                pss[q],
                                lhsT=lhs,
                                rhs=swT[:, 2 * kc : 2 * kc + 2, q * MM_N : (q + 1) * MM_N],
                                start=(kc == 0),
                                stop=(kc == KC - 1),
                                perf_mode=mybir.MatmulPerfMode.DoubleRow,
                            )
                    yt = youtp.tile([P, NW], f32)
                    for q in range(NQ):
                        nc.vector.scalar_tensor_tensor(
                            out=yt[:, q * MM_N : (q + 1) * MM_N],
                            in0=pss[q],
                            scalar=mean_sc[:, 0:1],
                            in1=meanb[:, s * NW + q * MM_N : s * NW + (q + 1) * MM_N],
                            op0=mybir.AluOpType.mult,
                            op1=mybir.AluOpType.add,
                        )
                    nc.sync.dma_start(
                        y[tb * P : (tb + 1) * P, s * NW : (s + 1) * NW], yt
                    )

    nc.compile()
    return nc


_CACHE = {}


def _get_nc(T, K, N, n_cores, n_tot):
    key = (T, K, N, n_cores, n_tot)
    if key not in _CACHE:
        _CACHE[key] = _build_nc(T, K, N, n_cores, n_tot)
    return _CACHE[key]


def kernel(x, W, b, n_cores=8, _trace=False):
    from concourse.bass_utils import run_bass_kernel_spmd

    x = np.ascontiguousarray(np.asarray(x, dtype=np.float32))
    W = np.ascontiguousarray(np.asarray(W, dtype=np.float32))
    b = np.ascontiguousarray(np.asarray(b, dtype=np.float32))

    tokens, k = x.shape
    n = W.shape[0]
    assert W.shape == (n, k) and b.shape == (n,)
    assert tokens % n_cores == 0
    tsh = tokens // n_cores

    nc = _get_nc(tsh, k, n, n_cores, tokens * k)

    wt = np.ascontiguousarray(W.T)  # [K, N] layout for the replicated weight
    b2 = b.reshape(1, n)
    in_maps = [
        {"x": np.ascontiguousarray(x[c * tsh : (c + 1) * tsh]), "WT": wt, "b": b2}
        for c in range(n_cores)
    ]
    res = run_bass_kernel_spmd(
        nc, in_maps, core_ids=list(range(n_cores)), trace=_trace
    )
    out = np.concatenate([r["y"] for r in res.results], axis=0)
    if _trace:
        kernel.last_results = res
    return out.astype(np.float32, copy=False)
